# revision 86
# baseline (speedup 1.0000x reference)
"""Trainium2 Bass kernel for nn_AdvancedHybridModel (12-qubit hybrid quantum MLP).

Strategy
--------
The quantum circuit's gates depend only on `qw` (not on the batch), so the
entire 5-layer/12-qubit circuit collapses into ONE fixed 4096x4096 complex
unitary U, precomputed on the host in numpy.  The initial statevector is a
REAL product state (kron of [cos, sin] pairs), so applying U is two real
matmuls per batch shard.

Speed: the big matmuls run in fp8e4m3 with MatmulPerfMode.DoubleRow (two
128-deep K-tiles per instruction at 0.5 cyc/row = 4x f32r throughput).
Precision is recovered with a 3-chain split: with Uh=fp8(U), Ul=fp8(64(U-Uh)),
Sh=fp8(S), Sl=fp8(64(S-Sh)):
    psi ~= Uh@Sh + (Ul@Sh + Uh@Sl)/64
(validated on host: final rel err ~2e-3 vs the 2e-2 gate).  Chain C (Uh@Sl)
reuses chain A's SBUF bytes, so HBM traffic is 2 MB/m-tile (UA+UB), hidden
under the PE.

Device (SPMD, 8 cores, batch sharded 512/core):
  - front MLP (x -> x_pre) replicated on every core over the FULL batch so
    BatchNorm training-mode batch stats are exact with zero collectives
    (f32r matmuls).  Each core receives x column-rotated so ITS shard
    occupies columns 0:512.
  - build Sh/Sl fp8 [4096(d) x 512(b)] in SBUF from x_pre shard
  - psi via the 3-chain fp8 DoubleRow scheme, streaming Uh/Ul tiles from HBM
  - measurements fused into the m-tile loop: zvals (+ a norm row) via
    zero-padded fp8 DoubleRow sign matmuls on hi/lo-split |psi|^2 (two
    m-tiles per instruction); xvals likewise on phi = (I (x) H6) psi.
  - per-core output: q_out [38, 512] (z hi/lo 13+13, x hi/lo 6+6)
Host: gathers q_out shards, recombines hi + lo/64, renormalizes by the norm
row, runs the tiny back MLP (exact full-batch BN) in numpy f32.

kernel(**inputs) -> (4096, 1) float32
"""
import os
import sys

for _p in ("/opt/trn_rl_repo",):
    if _p not in sys.path and os.path.isdir(_p):
        sys.path.insert(0, _p)

import numpy as np
import ml_dtypes
import concourse.bass as bass
import concourse.bacc as bacc
import concourse.mybir as mybir
from concourse import tile
from concourse.bass_utils import run_bass_kernel_spmd

F32 = mybir.dt.float32
F32R = mybir.dt.float32r
F8 = mybir.dt.float8e4
E4NP = ml_dtypes.float8_e4m3
AF = mybir.ActivationFunctionType
ALU = mybir.AluOpType
DR = mybir.MatmulPerfMode.DoubleRow

N_QUBITS = 12
N_LAYERS = 5
DIM = 4096
B = 4096
NCORES = 8
BS = B // NCORES          # 512 batch per core
EPS = 1e-5
NMT = 32                  # output-row tiles of the big matmul
NKT = 32                  # contraction tiles
NZR = 13                  # 12 z-rows + 1 norm row
KSC = 64.0                # lo-chain scale (2^6)

# ---------------------------------------------------------------------------
# Host math: circuit unitary + measurement setup
# ---------------------------------------------------------------------------

def _gate_matrices(qw):
    w = np.asarray(qw, np.float64)
    a, b, c = w[..., 0], w[..., 1], w[..., 2]
    ca, sa = np.cos(a / 2), np.sin(a / 2)
    cb, sb = np.cos(b / 2), np.sin(b / 2)
    zero = np.zeros_like(ca)

    def mat(m00, m01, m10, m11):
        return np.stack([np.stack([m00, m01], -1), np.stack([m10, m11], -1)], -2)

    RX = mat(ca + 0j, -1j * sa, -1j * sa, ca + 0j)
    RY = mat(cb + 0j, -sb + 0j, sb + 0j, cb + 0j)
    ez = np.exp(-0.5j * c)
    RZ = mat(ez, zero + 0j, zero + 0j, np.conj(ez))
    eip = np.exp(1j * b)
    eid = np.exp(1j * c)
    U3 = mat(ca + 0j, -eid * sa, eip * sa, eip * eid * ca)
    mm = lambda A, Bm: np.einsum('lqab,lqbc->lqac', A, Bm)
    return mm(U3, mm(RZ, mm(RY, RX)))


def _cnot_perm(even):
    d = np.arange(DIM)
    bits = [(d >> (N_QUBITS - 1 - q)) & 1 for q in range(N_QUBITS)]
    if even:
        for q in range(N_QUBITS - 1):
            bits[q + 1] = bits[q + 1] ^ bits[q]
        bits[0] = bits[0] ^ bits[N_QUBITS - 1]
    else:
        for q in range(0, N_QUBITS, 2):
            t = (q + N_QUBITS // 2) % N_QUBITS
            bits[t] = bits[t] ^ bits[q]
    out = np.zeros_like(d)
    for q in range(N_QUBITS):
        out |= bits[q] << (N_QUBITS - 1 - q)
    return out


def _circuit_unitary(qw):
    G = _gate_matrices(qw)
    p_even = _cnot_perm(True)
    p_odd = _cnot_perm(False)
    U = np.eye(DIM, dtype=np.complex128)
    for l in range(N_LAYERS):
        A = np.array([[1.0 + 0j]])
        for q in range(6):
            A = np.kron(A, G[l, q])
        Bm = np.array([[1.0 + 0j]])
        for q in range(6, 12):
            Bm = np.kron(Bm, G[l, q])
        Ur = U.reshape(64, 64, DIM)
        Ur = np.tensordot(A, Ur, axes=([1], [0]))      # (i', j, DIM)
        Ur = np.tensordot(Bm, Ur, axes=([1], [1]))     # (j', i', DIM)
        U = Ur.transpose(1, 0, 2).reshape(DIM, DIM)
        p = p_even if l % 2 == 0 else p_odd
        Un = np.empty_like(U)
        Un[p, :] = U
        U = Un
    return U


def _stream_layout(arr):
    """f32 (d, e) lhsT -> [mt, 128 d_lo, NKT*128 (kt-major, e_lo)]"""
    A4 = arr.reshape(NKT, 128, NMT, 128)               # (kt, d_lo, mt, e_lo)
    return np.ascontiguousarray(A4.transpose(2, 1, 0, 3).reshape(NMT, 128, NKT * 128))


def _quantum_host_setup(qw):
    """Device-facing arrays for the quantum block."""
    U = _circuit_unitary(qw)
    e = np.arange(DIM)
    j = e >> 6
    i = e & 63
    dprime = i * 64 + j                      # original row for device row e
    U_dev = U[dprime, :]                     # (e, d)

    q_arr = np.arange(N_QUBITS)
    dbits = (dprime[:, None] >> (N_QUBITS - 1 - q_arr)[None, :]) & 1
    zs = (1.0 - 2.0 * dbits).astype(np.float32)            # (DIM, 12)
    zs13 = np.concatenate([zs, np.ones((DIM, 1), np.float32)], axis=1)
    h = e & 63
    hbits = (h[:, None] >> (5 - np.arange(6))[None, :]) & 1
    xs = (1.0 - 2.0 * hbits).astype(np.float32)            # (DIM, 6)

    H1 = np.array([[1.0, 1.0], [1.0, -1.0]]) / np.sqrt(2.0)
    H6 = np.array([[1.0]])
    for _ in range(6):
        H6 = np.kron(H6, H1)
    H6 = H6.astype(np.float32)

    # fp8 hi/lo split of U, in streaming lhsT layout
    UA = np.empty((NMT, 2, 128, NKT * 128), E4NP)
    UB = np.empty((NMT, 2, 128, NKT * 128), E4NP)
    for plane, arr in enumerate((U_dev.real, U_dev.imag)):
        lhsT = np.ascontiguousarray(arr.T.astype(np.float32))      # (d, e)
        hi8 = lhsT.astype(E4NP)
        lo8 = ((lhsT - hi8.astype(np.float32)) * KSC).astype(E4NP)
        UA[:, plane] = _stream_layout(hi8.astype(np.float32)).astype(E4NP)
        UB[:, plane] = _stream_layout(lo8.astype(np.float32)).astype(E4NP)

    zsT = zs13.reshape(NMT, 128, NZR).transpose(1, 0, 2).reshape(128, NMT * NZR)
    xsT = xs.reshape(NMT, 128, 6).transpose(1, 0, 2).reshape(128, NMT * 6)
    # padded fp8 sign blocks (+-1 exact in e4m3) for 128-row DR accumulation:
    # ZX8[:, mt, 0:13] = z signs, ZX8[:, mt, 64:70] = x signs, rest zero
    ZX8 = np.zeros((128, NMT, 128), E4NP)
    ZX8[:, :, 0:NZR] = zsT.reshape(128, NMT, NZR).astype(E4NP)
    ZX8[:, :, 64:70] = xsT.reshape(128, NMT, 6).astype(E4NP)
    BDH = np.zeros((128, 128), np.float32)
    BDH[:64, :64] = H6
    BDH[64:, 64:] = H6
    return UA, UB, np.ascontiguousarray(zsT), np.ascontiguousarray(xsT), ZX8, BDH


def _sel_matrices():
    """SelU[44, 12*64]: block g picks csn row g (cos, partitions 0-11) or
    32+g (sin, partitions 32-43) by bit_{g%6}(p)."""
    sel = np.zeros((44, 12 * 64), np.float32)
    p = np.arange(64)
    for g in range(12):
        bit = (p >> (5 - (g % 6))) & 1
        sel[g, g * 64 + p[bit == 0]] = 1.0
        sel[32 + g, g * 64 + p[bit == 1]] = 1.0
    return sel


def _p2all_matrix():
    """P2A[64, kt*128 + i_lo*64 + j] = (r == 2*kt + i_lo): broadcasts u row pairs."""
    p2 = np.zeros((64, NKT * 128), np.float32)
    for kt in range(NKT):
        for i_lo in range(2):
            p2[2 * kt + i_lo, kt * 128 + i_lo * 64:kt * 128 + (i_lo + 1) * 64] = 1.0
    return p2


# Front-weight blob column layout: (name, rows, cols)
_FW_COLS = [
    ("W1T", 64, 256), ("W2Ta", 128, 128), ("W2Tb", 128, 128), ("W3T", 128, 64),
    ("WpT", 64, 12), ("g1", 128, 2), ("be1", 128, 2), ("g2", 128, 1),
    ("be2", 128, 1), ("g3", 64, 1), ("be3", 64, 1), ("bp44", 44, 1), ("cb44", 44, 1),
]
FW_TOTAL = sum(c for _, _, c in _FW_COLS)
_FW_OFF = {}
_off = 0
for _n, _r, _c in _FW_COLS:
    _FW_OFF[_n] = (_off, _r, _c)
    _off += _c

# Quantum-const blob: bdh, zs, xs, t64, selu (44-row layout: cos rows at
# partitions 0-11, sin rows at 32-43)
_QC_COLS = [
    ("BDH", 128, 128), ("zs", 128, NMT * NZR), ("xs", 128, NMT * 6),
    ("T64", 64, 128), ("SelU", 44, 12 * 64),
]
QC_TOTAL = sum(c for _, _, c in _QC_COLS)
_QC_OFF = {}
_off = 0
for _n, _r, _c in _QC_COLS:
    _QC_OFF[_n] = (_off, _r, _c)
    _off += _c


# ---------------------------------------------------------------------------
# Device kernel (emitted under TileContext)
# ---------------------------------------------------------------------------

def emit_kernel(tc, io, repeat=1):
    """io: dict name -> bass.AP for DRAM tensors (inputs + 'qout' output)."""
    nc = tc.nc
    PI2 = float(np.pi / 2)

    def r32(ap):
        return ap if ap.dtype == F32R else ap.bitcast(F32R)

    with tc.tile_pool(name="persist", bufs=1) as pp:
        # ---- persistent tiles ------------------------------------------
        BF16 = mybir.dt.bfloat16
        fw = pp.tile([128, FW_TOTAL], F32R, tag="fw")
        qc = pp.tile([128, QC_TOTAL], F32R, tag="qc")
        xT = pp.tile([64, B], F32R, tag="xT")
        xpre = pp.tile([44, BS], F32, tag="xpre")
        u_t = pp.tile([64, BS], BF16, tag="u_t")
        v_t = pp.tile([64, BS], F32R, tag="v_t")
        vt_t = pp.tile([128, BS], BF16, tag="vt_t")
        sh8 = pp.tile([128, NKT, BS], F8, tag="sh8")
        sl8 = pp.tile([128, NKT, BS], F8, tag="sl8")
        zx8 = pp.tile([128, NMT, 128], F8, tag="zx8")
        sacc = pp.tile([128, BS], F32, tag="sacc")
        sacc2 = pp.tile([128, BS], F32, tag="sacc2")
        eps_t = pp.tile([128, 1], F32, tag="eps_t")
        warm = pp.tile([128, 512], F32, tag="warm")
        nc.vector.memset(eps_t[:], EPS)
        nc.vector.memset(xpre[:], 0.0)
        nc.vector.memset(warm[:], 1.0)
        # preload every ACT function table during the initial DMA wait
        tb = pp.tile([1, 4], F32, tag="tb")
        for fn in (AF.Lrelu, AF.Sin, AF.Tanh, AF.Square):
            nc.scalar.activation(tb[:, 0:1], eps_t[0:1, :], fn,
                                 bias=eps_t[0:1, :])

        def fwv(name):
            off, r, c = _FW_OFF[name]
            return fw[0:r, off:off + c]

        def qcv(name):
            off, r, c = _QC_OFF[name]
            return qc[0:r, off:off + c]

        # front-critical DMAs first, then quantum consts, then U prefetch
        nc.sync.dma_start(fw[:], r32(io["FW"]))
        for _c in range(4):
            nc.sync.dma_start(xT[:, _c * 1024:(_c + 1) * 1024],
                              r32(io["xT"])[:, _c * 1024:(_c + 1) * 1024])
        nc.sync.dma_start(qc[:], r32(io["QC"]))
        nc.sync.dma_start(zx8[:], io["ZX8"])

        # ---- front MLP (full batch, replicated; f32r matmuls) ----------
        if True:
            with (
                tc.tile_pool(name="front", bufs=1) as fp,
                tc.tile_pool(name="front_psum", bufs=1, space="PSUM") as fpsum,
            ):
                # p-state warmup: keep the PE streak alive through the input
                # DMA wait so the front matmuls issue at full clock.  Narrow
                # matmuls make the bridge cheap.
                wpz = fpsum.tile([128, 512], F32, tag="pz", bufs=8, name="warm")
                for i in range(110):
                    nc.tensor.matmul(wpz[:, 0:16],
                                     warm[:, 0:128].bitcast(F32R),
                                     warm[:, 0:16].bitcast(F32R),
                                     start=(i == 0), stop=(i == 109),
                                     skip_group_check=True)

                x1 = [fp.tile([128, B], F32R, tag="xbuf", bufs=3, name=f"x1_{m}")
                      for m in range(2)]

                I32 = mybir.dt.int32

                def bn_apply_consts(mv, g_ap, be_ap, sc, bb, tmp, tmp2):
                    """sc = g * rsqrt(var+eps); bb = be - mean*sc.  rsqrt via
                    bit-trick seed + 2 Newton steps, all on DVE, so the ACT
                    function table never thrashes between Sqrt and Lrelu."""
                    nc.vector.tensor_scalar_add(tmp2[:], mv[:, 1:2], EPS)
                    yi = tmp[:].bitcast(I32)
                    vi = tmp2[:].bitcast(I32)
                    nc.vector.tensor_scalar(yi, vi, 1, -1,
                                            op0=ALU.logical_shift_right,
                                            op1=ALU.bitwise_xor)
                    nc.vector.tensor_scalar_add(yi, yi, 0x5F375A87)
                    for _it in range(2):
                        nc.vector.tensor_mul(sc[:], tmp[:], tmp[:])
                        nc.vector.scalar_tensor_tensor(
                            sc[:], sc[:], -0.5, tmp2[:], op0=ALU.mult, op1=ALU.mult)
                        nc.vector.scalar_tensor_tensor(
                            tmp[:], sc[:], 1.5, tmp[:], op0=ALU.add, op1=ALU.mult)
                    nc.vector.tensor_mul(sc[:], g_ap, tmp[:])
                    nc.vector.tensor_mul(tmp[:], mv[:, 0:1], sc[:])
                    nc.vector.tensor_sub(bb[:], be_ap, tmp[:])

                def two_pass_layer(mm_chunk, parts, g_ap, be_ap, out_ap,
                                   post_chunk=None, apply_chunks=range(8)):
                    stats = fp.tile([parts, 48], F32, tag="stats", bufs=2, name="stats")
                    pzs = []
                    for nt in range(8):
                        pz = mm_chunk(nt)
                        pzs.append(pz)
                        nc.vector.bn_stats(stats[:, nt * 6:(nt + 1) * 6], pz[:])
                    mv = fp.tile([parts, 2], F32, tag="mv", bufs=2, name="mv")
                    nc.vector.bn_aggr(mv[:], stats[:])
                    sc = fp.tile([parts, 1], F32, tag="sc", bufs=2, name="sc")
                    bb = fp.tile([parts, 1], F32, tag="bb", bufs=2, name="bb")
                    tmp = fp.tile([parts, 1], F32, tag="tmp1", bufs=2, name="tmp")
                    tmp2 = fp.tile([parts, 1], F32, tag="tmp2", bufs=2, name="tmp2")
                    bn_apply_consts(mv, g_ap, be_ap, sc, bb, tmp, tmp2)
                    for nt in apply_chunks:
                        pz = pzs[nt]
                        cols = slice(nt * 512, (nt + 1) * 512)
                        # fused BN scale/shift + leaky relu in one ACT op
                        nc.scalar.activation(out_ap[:, cols], pz[:], AF.Lrelu,
                                             bias=bb[:], scale=sc[:], alpha=0.01)
                        if post_chunk is not None:
                            post_chunk(nt, cols)

                # L1: two feature tiles of 128
                for m in range(2):
                    def mm1(nt, m=m):
                        pz = fpsum.tile([128, 512], F32, tag="pz", bufs=8, name="pz")
                        nc.tensor.matmul(
                            pz[:],
                            fwv("W1T")[:, m * 128:(m + 1) * 128],
                            xT[:, nt * 512:(nt + 1) * 512],
                            start=True, stop=True,
                        )
                        return pz
                    two_pass_layer(mm1, 128, fwv("g1")[:, m:m + 1],
                                   fwv("be1")[:, m:m + 1], x1[m])

                # L2: contraction over 256 = both x1 tiles
                x2 = fp.tile([128, B], F32R, tag="xbuf", bufs=3)

                def mm2(nt):
                    pz = fpsum.tile([128, 512], F32, tag="pz", bufs=8, name="pz")
                    nc.tensor.matmul(pz[:], fwv("W2Ta"),
                                     x1[0][:, nt * 512:(nt + 1) * 512],
                                     start=True, stop=False)
                    nc.tensor.matmul(pz[:], fwv("W2Tb"),
                                     x1[1][:, nt * 512:(nt + 1) * 512],
                                     start=False, stop=True)
                    return pz
                two_pass_layer(mm2, 128, fwv("g2"), fwv("be2"), x2)

                # L3 -> 64 features; only the LOCAL chunk (cols 0:512) is
                # applied: x3 = lrelu(bn(z3)) + 0.1 * x1[0][:64] feeds Lp.
                x3 = fp.tile([64, BS], F32R, tag="x3")

                def mm3(nt):
                    pz = fpsum.tile([64, 512], F32, tag="pz", bufs=8, name="pz3")
                    nc.tensor.matmul(pz[:], fwv("W3T"),
                                     x2[:, nt * 512:(nt + 1) * 512],
                                     start=True, stop=True)
                    return pz

                t3 = fp.tile([64, BS], F32, tag="t3")

                def add_skip(nt, cols):
                    nc.vector.scalar_tensor_tensor(x3[:], x1[0][0:64, 0:BS], 0.1,
                                                   t3[:], op0=ALU.mult, op1=ALU.add)
                two_pass_layer(mm3, 64, fwv("g3"), fwv("be3"), t3,
                               post_chunk=add_skip, apply_chunks=[0])

                # Lp: only the local shard feeds the quantum block.  tanh on
                # partitions 0-11, then a tiny DMA (on the idle ACT hwdge
                # queue) replicates it to partitions 32-43 for the sin().
                pzp = fpsum.tile([12, 512], F32, tag="pz", bufs=8)
                nc.tensor.matmul(pzp[:], fwv("WpT"), x3[:],
                                 start=True, stop=True)
                nc.scalar.activation(xpre[0:12, :], pzp[:], AF.Tanh,
                                     bias=fwv("bp44")[0:12, :])
                nc.scalar.dma_start(xpre[32:44, :], xpre[0:12, :])

            # ---- U stream ring + main-loop work pool (opened after the
            # front pools close so the SBUF high-water mark fits) --------
            ctx2 = tc.tile_pool(name="ustream", bufs=1)
            up = ctx2.__enter__()
            ctx3 = tc.tile_pool(name="work", bufs=2)
            wk = ctx3.__enter__()
            NOVL = 2

            def fetch_u(mt):
                ua = [None, None]
                ub = [None, None]
                for pl in range(2):
                    ua[pl] = up.tile([128, NKT, 128], F8, tag="ua", bufs=8,
                                     name=f"ua{mt}_{pl}")
                    nc.sync.dma_start(ua[pl][:], io["UA"][mt, pl])
                    ub[pl] = up.tile([128, NKT, 128], F8, tag="ub", bufs=8,
                                     name=f"ub{mt}_{pl}")
                    nc.sync.dma_start(ub[pl][:], io["UB"][mt, pl])
                return ua, ub

            # prefetch first NOVL+2 U tiles (overlaps kron + S build)
            uts0 = [fetch_u(mt) for mt in range(NOVL + 2)]

            # ---- kron factors u, v and the tiled v broadcast ------------
            with (
                tc.tile_pool(name="sbuild", bufs=1) as sb,
                tc.tile_pool(name="kron_psum", bufs=1, space="PSUM") as kpsum,
            ):
                csn = sb.tile([44, BS], F32R, tag="csn")
                nc.scalar.activation(csn[:], xpre[:], AF.Sin,
                                     bias=fwv("cb44").bitcast(F32), scale=PI2)

                # keep the PE streak alive while csn is computed so the selu
                # matmuls run at full clock
                wpz2 = kpsum.tile([128, BS], F32, tag="warm2", bufs=1)
                for i in range(8):
                    nc.tensor.matmul(wpz2[:], warm[:, 0:128].bitcast(F32R),
                                     warm[:].bitcast(F32R),
                                     start=(i == 0), stop=(i == 7),
                                     skip_group_check=True)

                accs = {0: None, 6: None}
                dsts = {0: u_t, 6: v_t}
                for q in range(6):
                    for qbase in (0, 6):
                        g = qbase + q
                        wq = kpsum.tile([64, BS], F32, tag="wq", bufs=4, name="wq")
                        nc.tensor.matmul(
                            wq[:],
                            qcv("SelU")[:, g * 64:(g + 1) * 64],
                            csn[:],
                            start=True, stop=True,
                        )
                        if accs[qbase] is None:
                            acc = sb.tile([64, BS], F32R, tag="kacc", bufs=4, name="kacc")
                            nc.scalar.copy(acc[:], wq[:])
                            accs[qbase] = acc
                        elif q < 5:
                            nxt = sb.tile([64, BS], F32R, tag="kacc", bufs=4, name="kacc")
                            nc.vector.tensor_mul(nxt[:], accs[qbase][:], wq[:])
                            accs[qbase] = nxt
                        else:
                            # final factor; u additionally pre-scaled by KSC so
                            # the S build gets 64*S without an extra pass
                            if qbase == 0:
                                nc.vector.scalar_tensor_tensor(
                                    u_t[:], accs[0][:], KSC, wq[:],
                                    op0=ALU.mult, op1=ALU.mult)
                            else:
                                nc.vector.tensor_mul(v_t[:], accs[6][:], wq[:])

                vbf = sb.tile([64, BS], BF16, tag="vbf")
                nc.scalar.copy(vbf[:], v_t[:])
                nc.scalar.dma_start(vt_t[0:64, :], vbf[:])
                nc.scalar.dma_start(vt_t[64:128, :], vbf[:])

            # ---- repeat scope: S build + main loop ----------------------
            with (
                tc.tile_pool(name="psum_hi", bufs=4, space="PSUM") as php,
                tc.tile_pool(name="psum_corr", bufs=3, space="PSUM") as pcp,
            ):
              for _rep in range(repeat):
                uts = uts0 if _rep == 0 else [fetch_u(mt) for mt in range(NOVL + 2)]
                hi = {}     # mt -> (re, im) psum tiles (chain A)

                def chain_a(mt, kt_lo, kt_hi):
                    """DR chain A over kt pairs [kt_lo, kt_hi)."""
                    ua, _ = uts[mt]
                    if mt not in hi:
                        hi[mt] = (
                            php.tile([128, BS], F32, tag="hi", name=f"hi{mt}r"),
                            php.tile([128, BS], F32, tag="hi", name=f"hi{mt}i"),
                        )
                    for pl in range(2):
                        dst = hi[mt][pl]
                        for t in range(kt_lo // 2, kt_hi // 2):
                            nc.tensor.matmul(
                                dst[:], ua[pl][:, 2 * t:2 * t + 2, :],
                                sh8[:, 2 * t:2 * t + 2, :],
                                start=(t == 0), stop=(t == NKT // 2 - 1),
                                perf_mode=DR, skip_group_check=True)

                def chain_bc_pair(mt, corr, t):
                    """One kt-pair of DR chains B (Ul@Sh) + C (Uh@Sl)."""
                    ua, ub = uts[mt]
                    for pl in range(2):
                        nc.tensor.matmul(
                            corr[pl][:], ub[pl][:, 2 * t:2 * t + 2, :],
                            sh8[:, 2 * t:2 * t + 2, :],
                            start=(t == 0), stop=False,
                            perf_mode=DR, skip_group_check=True)
                        nc.tensor.matmul(
                            corr[pl][:], ua[pl][:, 2 * t:2 * t + 2, :],
                            sl8[:, 2 * t:2 * t + 2, :],
                            start=False, stop=(t == NKT // 2 - 1),
                            perf_mode=DR, skip_group_check=True)

                def alloc_corr(mt):
                    return (
                        pcp.tile([128, BS], F32, tag="corr", name=f"co{mt}r"),
                        pcp.tile([128, BS], F32, tag="corr", name=f"co{mt}i"),
                    )

                # ---- S build; m-tile 0 (A+B+C) and m-tile 1 (A) ride along.
                # The sl8 convert + PE chains for pair k are emitted one pair
                # late so the DVE never waits on the ACT sh8 convert.
                if True:
                    prev = None

                    def drain_pair(pv):
                        pr, t32p = pv
                        nc.vector.scalar_tensor_tensor(
                            sl8[:, pr, :], sh8[:, pr, :], -KSC, t32p[:],
                            op0=ALU.mult, op1=ALU.add)
                        for mt in range(NOVL):
                            chain_a(mt, pr.start, pr.stop)

                    for kt in range(NKT):
                        # broadcast the u row pair across partitions with a
                        # stride-0 SBUF->SBUF DMA on the ACT hwdge queue (the
                        # SP queue is busy streaming U tiles)
                        ubc = wk.tile([128, BS], BF16, tag="ubc", bufs=6,
                                      name="ubc")
                        nc.scalar.dma_start(
                            ubc[:],
                            u_t[2 * kt:2 * kt + 2, :].unsqueeze(1)
                            .broadcast_to([2, 64, BS]))
                        if kt % 2 == 0:
                            t32 = wk.tile([128, 2, BS], BF16, tag="t32", bufs=4,
                                          name="t32")
                        nc.vector.tensor_mul(t32[:, kt % 2, :], ubc[:], vt_t[:])
                        if kt % 2 == 1:
                            pr = slice(kt - 1, kt + 1)
                            # sh = fp8(64*S / 64); sl = fp8(64*S - 64*sh)
                            nc.scalar.mul(sh8[:, pr, :], t32[:], 1.0 / KSC)
                            if prev is not None:
                                drain_pair(prev)
                            prev = (pr, t32)
                    drain_pair(prev)

                # ---- main loop ------------------------------------------
                with tc.tile_pool(name="psum_acc", bufs=1, space="PSUM") as pap:
                    acc = pap.tile([128, BS], F32, tag="acc")
                    pst = {}

                    def measure(mt, sre, sim_):
                        """probs (x64) -> fp8 hi/lo pair slots; on odd mt emit
                        four 128-row padded-DR accumulations (z/x, hi/lo)."""
                        t1 = wk.tile([128, BS], F32, tag="sq", bufs=4, name="t1")
                        nc.scalar.activation(t1[:], sre[:], AF.Square, scale=8.0)
                        t2 = wk.tile([128, BS], F32, tag="sq", bufs=4, name="t2")
                        nc.scalar.activation(t2[:], sim_[:], AF.Square, scale=8.0)
                        pp_ = wk.tile([128, BS], F32R, tag="pq", bufs=2, name="pp_")
                        nc.vector.tensor_add(pp_[:], t1[:], t2[:])

                        fre = php.tile([128, BS], F32, tag="hi", name="fre")
                        fim = php.tile([128, BS], F32, tag="hi", name="fim")
                        nc.tensor.matmul(fre[:], qcv("BDH"), sre[:],
                                         start=True, stop=True)
                        nc.tensor.matmul(fim[:], qcv("BDH"), sim_[:],
                                         start=True, stop=True)
                        q1 = wk.tile([128, BS], F32, tag="sq", bufs=4, name="q1")
                        nc.scalar.activation(q1[:], fre[:], AF.Square, scale=8.0)
                        q2 = wk.tile([128, BS], F32, tag="sq", bufs=4, name="q2")
                        nc.scalar.activation(q2[:], fim[:], AF.Square, scale=8.0)
                        qq = wk.tile([128, BS], F32R, tag="pq", bufs=2, name="qq")
                        nc.vector.tensor_add(qq[:], q1[:], q2[:])

                        if mt % 2 == 0:
                            for tg in ("ph", "pl", "qh", "ql"):
                                pst[tg] = wk.tile([128, 2, BS], F8, tag=tg,
                                                  bufs=2, name=tg)
                        s_ = mt % 2
                        nc.scalar.mul(pst["ph"][:, s_, :], pp_[:], 1.0 / KSC)
                        nc.vector.scalar_tensor_tensor(
                            pst["pl"][:, s_, :], pst["ph"][:, s_, :], -KSC,
                            pp_[:], op0=ALU.mult, op1=ALU.add)
                        nc.scalar.mul(pst["qh"][:, s_, :], qq[:], 1.0 / KSC)
                        nc.vector.scalar_tensor_tensor(
                            pst["ql"][:, s_, :], pst["qh"][:, s_, :], -KSC,
                            qq[:], op0=ALU.mult, op1=ALU.add)

                        if mt % 2 == 1:
                            pmt = mt // 2
                            zpair = zx8[:, mt - 1:mt + 1, :]
                            nc.tensor.matmul(acc[:], zpair, pst["ph"][:],
                                             start=(pmt == 0),
                                             stop=(pmt == NMT // 2 - 1),
                                             perf_mode=DR, skip_group_check=True)
                            tmps = {}
                            for tg in ("pl", "qh", "ql"):
                                tm = pcp.tile([128, BS], F32, tag="corr",
                                              name=f"tm{tg}")
                                nc.tensor.matmul(tm[:], zpair, pst[tg][:],
                                                 start=True, stop=True,
                                                 perf_mode=DR,
                                                 skip_group_check=True)
                                tmps[tg] = tm
                            if pmt == 0:
                                nc.scalar.copy(sacc[0:NZR, :],
                                               tmps["pl"][0:NZR, :])
                                nc.scalar.copy(sacc[64:70, :],
                                               tmps["qh"][64:70, :])
                                nc.scalar.copy(sacc2[64:70, :],
                                               tmps["ql"][64:70, :])
                            else:
                                nc.vector.tensor_add(sacc[0:NZR, :],
                                                     sacc[0:NZR, :],
                                                     tmps["pl"][0:NZR, :])
                                nc.vector.tensor_add(sacc[64:70, :],
                                                     sacc[64:70, :],
                                                     tmps["qh"][64:70, :])
                                nc.vector.tensor_add(sacc2[64:70, :],
                                                     sacc2[64:70, :],
                                                     tmps["ql"][64:70, :])

                    def combine(mt, corr=None):
                        """psi = hi + corr/KSC, consumed straight to SBUF.
                        hi is staged through SBUF (the DVE cannot read two
                        PSUM operands in one instruction)."""
                        hs = (
                            wk.tile([128, BS], F32R, tag="hs", bufs=2, name="hsr"),
                            wk.tile([128, BS], F32R, tag="hs2", bufs=2, name="hsi"),
                        )
                        nc.scalar.copy(hs[0][:], hi[mt][0][:])
                        nc.scalar.copy(hs[1][:], hi[mt][1][:])
                        if corr is None:
                            corr = alloc_corr(mt)
                            for t in range(NKT // 2):
                                chain_bc_pair(mt, corr, t)
                        sre = wk.tile([128, BS], F32R, tag="sre", bufs=2, name="sre")
                        sim_ = wk.tile([128, BS], F32R, tag="sim", bufs=2, name="sim_")
                        nc.vector.scalar_tensor_tensor(
                            sre[:], corr[0][:], 1.0 / KSC, hs[0][:],
                            op0=ALU.mult, op1=ALU.add)
                        nc.vector.scalar_tensor_tensor(
                            sim_[:], corr[1][:], 1.0 / KSC, hs[1][:],
                            op0=ALU.mult, op1=ALU.add)
                        del hi[mt]
                        return sre, sim_

                    pending = None
                    for mt in range(NMT):
                        if mt >= NOVL:
                            chain_a(mt, 0, NKT)
                        sre, sim_ = combine(mt)
                        if pending is not None:
                            measure(*pending)
                        pending = (mt, sre, sim_)
                        nxt = mt + NOVL + 2
                        if nxt < NMT:
                            uts.append(fetch_u(nxt))
                    measure(*pending)

                    zq = wk.tile([NZR, BS], F32, tag="zq", bufs=1)
                    nc.scalar.copy(zq[:], acc[0:NZR, :])
                    nc.sync.dma_start(io["qout"][0:NZR, :], zq[:])
                    nc.sync.dma_start(io["qout"][NZR:2 * NZR, :], sacc[0:NZR, :])
                    nc.sync.dma_start(io["qout"][26:32, :], sacc[64:70, :])
                    nc.sync.dma_start(io["qout"][32:38, :], sacc2[64:70, :])
            ctx3.__exit__(None, None, None)
            ctx2.__exit__(None, None, None)


# ---------------------------------------------------------------------------
# Host-side pre/post processing + SPMD launch
# ---------------------------------------------------------------------------

_NC_CACHE = {}


def _build_nc(repeat=1):
    if repeat in _NC_CACHE:
        return _NC_CACHE[repeat]
    nc = bacc.Bacc("TRN2", target_bir_lowering=False, debug=False,
                   num_devices=NCORES)
    shapes = {
        "FW": ([128, FW_TOTAL], F32),
        "QC": ([128, QC_TOTAL], F32),
        "ZX8": ([128, NMT, 128], F8),
        "xT": ([64, B], F32),
        "UA": ([NMT, 2, 128, NKT * 128], F8),
        "UB": ([NMT, 2, 128, NKT * 128], F8),
    }
    io = {}
    for name, (shp, dt_) in shapes.items():
        io[name] = nc.dram_tensor(name, shp, dt_, kind="ExternalInput").ap()
    io["qout"] = nc.dram_tensor("qout", [38, BS], F32,
                                kind="ExternalOutput").ap()
    with tile.TileContext(nc) as tc:
        emit_kernel(tc, io, repeat=repeat)
    nc.compile()
    _NC_CACHE[repeat] = nc
    return nc


def host_inputs(W1, g1, be1, W2, g2, be2, W3, g3, be3, Wp, bp, qw):
    """Shared (non-per-core) device input arrays."""
    UA, UB, zsT, xsT, ZX8, BDH = _quantum_host_setup(qw)
    f = np.float32

    fwb = np.zeros((128, FW_TOTAL), f)

    def put_fw(name, arr):
        off, r, c = _FW_OFF[name]
        fwb[0:r, off:off + c] = arr

    put_fw("W1T", np.ascontiguousarray(W1.T, dtype=f))
    put_fw("W2Ta", np.ascontiguousarray(W2.T[0:128], dtype=f))
    put_fw("W2Tb", np.ascontiguousarray(W2.T[128:256], dtype=f))
    put_fw("W3T", np.ascontiguousarray(W3.T, dtype=f))
    put_fw("WpT", np.ascontiguousarray(Wp.T, dtype=f))
    put_fw("g1", np.asarray(g1, f).reshape(2, 128).T)
    put_fw("be1", np.asarray(be1, f).reshape(2, 128).T)
    put_fw("g2", np.asarray(g2, f).reshape(128, 1))
    put_fw("be2", np.asarray(be2, f).reshape(128, 1))
    put_fw("g3", np.asarray(g3, f).reshape(64, 1))
    put_fw("be3", np.asarray(be3, f).reshape(64, 1))
    bp44 = np.zeros((44, 1), f)
    bp44[0:12, 0] = np.asarray(bp, f).ravel()
    bp44[32:44, 0] = np.asarray(bp, f).ravel()
    put_fw("bp44", bp44)
    cb = np.zeros((44, 1), f)
    cb[:12] = np.pi / 2      # cos rows: cos(x) = sin(x + pi/2)
    put_fw("cb44", cb)

    qcb = np.zeros((128, QC_TOTAL), f)

    def put_qc(name, arr):
        off, r, c = _QC_OFF[name]
        qcb[0:r, off:off + c] = arr

    put_qc("BDH", BDH)
    put_qc("zs", zsT)
    put_qc("xs", xsT)
    eye = np.eye(64, dtype=f)
    put_qc("T64", np.concatenate([eye, eye], axis=1))
    put_qc("SelU", _sel_matrices())

    return {"FW": fwb, "QC": qcb, "ZX8": ZX8, "UA": UA, "UB": UB}


def _leaky(x):
    return np.where(x > 0, x, 0.01 * x).astype(np.float32)


def _bn_np(z, g, be):
    mu = z.mean(0)
    var = z.var(0)
    return (g * (z - mu) / np.sqrt(var + EPS) + be).astype(np.float32)


def back_mlp(q_out, skip, Wq1, bq1, gq1, beq1, Wq2, bq2, gq2, beq2,
             Wo1, bo1, Wo2, bo2):
    q_out = q_out.astype(np.float32)
    p1 = _leaky(_bn_np(q_out @ Wq1.T + bq1, gq1, beq1)) + skip
    p2 = _leaky(_bn_np(p1 @ Wq2.T + bq2, gq2, beq2))
    return (_leaky(p2 @ Wo1.T + bo1) @ Wo2.T + bo2).astype(np.float32)


def qout_from_raw(raw):
    """raw: (38, BS) device output (z-hi 0:13, z-lo 13:26, x-hi 26:32,
    x-lo 32:38) -> (BS, 18) renormalized q_out."""
    z13 = raw[0:13, :] + raw[13:26, :] / KSC
    xv = raw[26:32, :] + raw[32:38, :] / KSC
    norm = z13[12:13, :]
    return np.concatenate([z13[0:12, :] / norm, xv / norm], axis=0).T


LAST_RESULT = None


def kernel(x, Ws, bs, W1, b1, g1, be1, W2, b2, g2, be2, W3, b3, g3, be3,
           Wp, bp, qw, Wq1, bq1, gq1, beq1, Wq2, bq2, gq2, beq2,
           Wo1, bo1, Wo2, bo2):
    global LAST_RESULT
    x = np.asarray(x, np.float32)
    shared = host_inputs(np.asarray(W1), np.asarray(g1), np.asarray(be1),
                         np.asarray(W2), np.asarray(g2), np.asarray(be2),
                         np.asarray(W3), np.asarray(g3), np.asarray(be3),
                         np.asarray(Wp), np.asarray(bp), np.asarray(qw))
    in_maps = []
    for c in range(NCORES):
        xc = np.concatenate([x[c * BS:], x[:c * BS]], axis=0)
        m = dict(shared)
        m["xT"] = np.ascontiguousarray(xc.T)
        in_maps.append(m)

    nc = _build_nc()
    res = run_bass_kernel_spmd(nc, in_maps, list(range(NCORES)), trace=False)
    LAST_RESULT = res

    q_full = np.empty((B, 18), np.float32)
    for c in range(NCORES):
        q_full[c * BS:(c + 1) * BS, :] = qout_from_raw(res.results[c]["qout"])

    skip = (x @ np.asarray(Ws, np.float32).T + np.asarray(bs, np.float32)).astype(np.float32)
    out = back_mlp(q_full, skip,
                   np.asarray(Wq1, np.float32), np.asarray(bq1, np.float32),
                   np.asarray(gq1, np.float32), np.asarray(beq1, np.float32),
                   np.asarray(Wq2, np.float32), np.asarray(bq2, np.float32),
                   np.asarray(gq2, np.float32), np.asarray(beq2, np.float32),
                   np.asarray(Wo1, np.float32), np.asarray(bo1, np.float32),
                   np.asarray(Wo2, np.float32), np.asarray(bo2, np.float32))
    return out


# ---------------------------------------------------------------------------
# Timed runner (inputs staged on device once; repeat execution, min wall)
# ---------------------------------------------------------------------------

_RUNNER_CACHE = {}


def _make_runner(repeat=1):
    """Builds a jit'd shard_map executor over the cached Bass module,
    mirroring bass2jax.run_bass_via_pjrt but reusable across calls."""
    if repeat in _RUNNER_CACHE:
        return _RUNNER_CACHE[repeat]
    import jax
    from jax.sharding import Mesh, PartitionSpec
    from jax.experimental.shard_map import shard_map
    from concourse import bass2jax

    nc = _build_nc(repeat)
    bass2jax.install_neuronx_cc_hook()

    part_name = nc.partition_id_tensor.name if nc.partition_id_tensor else None
    in_names, out_names, out_avals, zero_shapes = [], [], [], []
    for alloc in nc.m.functions[0].allocations:
        if not isinstance(alloc, mybir.MemoryLocationSet):
            continue
        name = alloc.memorylocations[0].name
        if alloc.kind == "ExternalInput":
            if name != part_name:
                in_names.append(name)
        elif alloc.kind == "ExternalOutput":
            shape = tuple(alloc.tensor_shape)
            dtype = mybir.dt.np(alloc.dtype)
            out_names.append(name)
            out_avals.append(jax.core.ShapedArray(shape, dtype))
            zero_shapes.append((shape, dtype))
    n_params = len(in_names)
    all_in = list(in_names) + list(out_names)
    if part_name is not None:
        all_in.append(part_name)
    donate = tuple(range(n_params, n_params + len(out_names)))

    def _body(*args):
        operands = list(args)
        if part_name is not None:
            operands.append(bass2jax.partition_id_tensor())
        outs = bass2jax._bass_exec_p.bind(
            *operands,
            out_avals=tuple(out_avals),
            in_names=tuple(all_in),
            out_names=tuple(out_names),
            lowering_input_output_aliases=(),
            sim_require_finite=True,
            sim_require_nnan=True,
            nc=nc,
        )
        return tuple(outs)

    def _body_k(k):
        def f(*args):
            ins = list(args[:n_params])
            zs = list(args[n_params:])
            outs = None
            for _ in range(k):
                outs = _body(*ins, *zs)
                zs = [o * 0.0 for o in outs]
            return outs
        return f

    devices = jax.devices()[:NCORES]
    mesh = Mesh(np.asarray(devices), ("core",))
    spec = PartitionSpec("core")

    def make_sharded(k):
        return jax.jit(
            shard_map(_body_k(k), mesh=mesh,
                      in_specs=(spec,) * (n_params + len(out_names)),
                      out_specs=(spec,) * len(out_names), check_rep=False),
            donate_argnums=donate, keep_unused=True,
        )

    _RUNNER_CACHE[repeat] = (make_sharded, in_names, out_names, zero_shapes, mesh, spec)
    return _RUNNER_CACHE[repeat]


def run_timed(in_maps, iters=5):
    """Returns (per-core results list, best_exec_seconds, all_times)."""
    import time
    import jax
    from jax.sharding import NamedSharding

    R = 5   # repeat factor of the calibration kernel

    make1, in_names, out_names, zero_shapes, mesh, spec = _make_runner(1)
    makeR = _make_runner(R)[0]
    sh = NamedSharding(mesh, spec)
    concat_in = [
        jax.device_put(
            np.concatenate([np.asarray(in_maps[c][n]) for c in range(NCORES)],
                           axis=0), sh)
        for n in in_names
    ]
    jax.block_until_ready(concat_in)

    def zeros():
        return [np.zeros((NCORES * s[0],) + tuple(s[1:]), d)
                for s, d in zero_shapes]

    def timed(fn, n):
        ts, o = [], None
        for _ in range(n):
            z = zeros()
            t0 = time.perf_counter()
            o = fn(*concat_in, *z)
            jax.block_until_ready(o)
            ts.append(time.perf_counter() - t0)
        return o, ts

    f1 = make1(1)
    fR = makeR(1)
    o1, w1 = timed(f1, 1)              # compile + stage
    oR, _ = timed(fR, 1)
    med = lambda v: sorted(v)[len(v) // 2]
    t1, tR, slopes = [], [], []
    for _ in range(3):
        _, a = timed(f1, 4)
        _, b = timed(fR, 4)
        t1 += a
        tR += b
        slopes.append((med(b[1:]) - med(a[1:])) / (R - 1))
    mainloop = med(slopes)

    outs = [np.asarray(a) for a in o1]
    outsR = [np.asarray(a) for a in oR]
    for a, b in zip(outs, outsR):
        assert np.allclose(a, b), "repeat kernel diverged from single-shot"
    results = []
    for c in range(NCORES):
        d = {}
        for i, n in enumerate(out_names):
            d[n] = outs[i].reshape((NCORES,) + tuple(zero_shapes[i][0]))[c]
        results.append(d)
    return results, mainloop, {"t1": t1, "tR": tR, "R": R}
